# revision 25
# baseline (speedup 1.0000x reference)
"""BitLinear (ternary-weight + int8-activation quantized linear) on 8 Trainium2
NeuronCores, column-parallel over out_features.

Contract: kernel(x, weight) with x (2, 2048, 4096) f32, weight (16384, 4096) f32
returns (2, 2048, 16384) f32 — the full unsharded output.

Strategy
--------
- Shard weight rows (out_features) 8 ways; replicate x (the sharding hint).
- The quantized GEMM is exact integer math: |x_q| <= 127 fits bf16 exactly and
  the ternary weights {-1,0,+1} fit fp8e4m3 exactly, so a bf16(stationary) x
  fp8(moving) matmul with fp32 PSUM accumulation reproduces it bit-exactly;
  all scales fold into an fp32 epilogue (gamma * scale_w / 127 per token).
- Schedule (the tensor engine's 874us of bf16 matmul is the roofline; keep it
  fed): pass 1 abs-sums the fp32 weight slice with DMA split across the two
  HWDGE queues (scalar+sync) and reductions split vector/gpsimd, then a 4-byte
  AllReduce; the x-side quantization of the first NPREF token tiles overlaps
  the collective's latency.  Pass 2 re-reads the slice og-major (issues on the
  gpsimd soft-DGE queue so pool throttling never blocks an urgent queue),
  ternarizes scalar+vector, DMA-xbar transposes bf16, and vector-casts into a
  single resident fp8 slab [128, c, h, k, o] whose matmul view is a strided
  [128, 4, 128] moving AP — no slow gpsimd CAST on the critical path.
- Matmuls run og-outer over the NPREF prefetched token tiles while pass 2
  finishes the later groups, then t-outer; per-token scale applied on the
  PSUM->SBUF eviction (scalar), output stores on the scalar queue so the sync
  queue stays dedicated to x loads + transposes.
"""

import sys

sys.path.insert(0, "/opt/trn_rl_repo")

import numpy as np

import concourse.bass as bass
import concourse.mybir as mybir
import concourse.tile as tile

import bass_rust
from concourse.bass_utils import run_bass_kernel_spmd

F32 = mybir.dt.float32
BF16 = mybir.dt.bfloat16
FP8 = mybir.dt.float8e4
CMAGIC = 12582912.0  # 2^23 + 2^22: (v + C) - C == round-half-even(v), |v| < 2^22
EPS = 1e-8

N_CORES = 8
B, T, D_IN, D_OUT = 2, 2048, 4096, 16384
TOK = B * T                      # 4096 tokens
OPC = D_OUT // N_CORES           # 2048 out features per core
NTOK = TOK // 128                # 32 token tiles
ND = D_IN // 128                 # 32 contraction tiles
NWC = OPC // 128                 # 16 weight row chunks per core
NOG = OPC // 512                 # 4 output groups
DH = D_IN // 2                   # 2048 staging width
NDH = DH // 128                  # 16 d-tiles per half
CW = 1024                        # weight pipeline chunk width (d)
NJ = D_IN // CW                  # 4 chunk columns
NKJ = CW // 128                  # 8 k-tiles per chunk
NPREF = 4                        # token tiles prepped before the matmul phase


def _split_multi_waits(nc):
    """This container's walrus build rejects >1 sync wait per instruction, but
    Tile emits multi-wait instructions. Move extra waits onto preceding
    single-wait NoOps on the same engine (identical blocking semantics)."""
    wid = 0
    for f in nc.m.functions:
        for blk in f.blocks:
            insts = list(blk.instructions)
            new = []
            changed = False
            for inst in insts:
                si = inst.sync_info
                if si is not None and len(si.on_wait) > 1:
                    waits = list(si.on_wait)
                    for w in waits[:-1]:
                        nop = mybir.InstNoOp(name=f"WSPLIT-{wid}", ins=[], outs=[])
                        wid += 1
                        nop.engine = inst.engine
                        nop.sync_info = bass_rust.SyncInfo(on_wait=[w], on_update=[])
                        new.append(nop)
                    inst.sync_info = bass_rust.SyncInfo(
                        on_wait=[waits[-1]], on_update=list(si.on_update)
                    )
                    changed = True
                new.append(inst)
            if changed:
                blk.instructions = new


def build_bitlinear_nc():
    nc = bass.Bass("TRN2", target_bir_lowering=False, debug=False,
                   num_devices=N_CORES)
    x_d = nc.dram_tensor("x", [TOK, D_IN], F32, kind="ExternalInput")
    w_d = nc.dram_tensor("weight", [OPC, D_IN], F32, kind="ExternalInput")
    out_d = nc.dram_tensor("out", [TOK, OPC], F32, kind="ExternalOutput")
    cc_buf = nc.dram_tensor("cc_buf", [1, 1], F32)
    cc_warm = nc.dram_tensor("cc_warm", [1, 1], F32)
    sc2_dram = nc.dram_tensor("sc2_d", [1, 2], F32)

    with tile.TileContext(nc, trace_sim=False) as tc:
        with (
            tc.tile_pool(name="wT", bufs=1) as wT_pool,
            tc.tile_pool(name="w32", bufs=10) as w32_pool,
            tc.tile_pool(name="x32", bufs=3) as x32_pool,
            tc.tile_pool(name="wt1", bufs=2) as wt1_pool,
            tc.tile_pool(name="xt1", bufs=2) as xt1_pool,
            tc.tile_pool(name="wtern", bufs=3) as wtern_pool,
            tc.tile_pool(name="wtT", bufs=3) as wtT_pool,
            tc.tile_pool(name="xq16", bufs=2) as xq16_pool,
            tc.tile_pool(name="xqT", bufs=NPREF) as xqT_pool,
            tc.tile_pool(name="outs", bufs=3) as outs_pool,
            tc.tile_pool(name="small", bufs=1) as small,
            tc.tile_pool(name="psum", bufs=2, space="PSUM") as psum_pool,
        ):
            # resident ternary weight, per og: [d % 128, j, k-in-j, o-in-group]
            w8 = [wT_pool.tile([128, NJ, NKJ, 512], FP8, tag=f"w8_{g}",
                               name=f"w8_{g}")
                  for g in range(NOG)]
            partials = small.tile([128, NJ * NWC], F32)
            cmag = small.tile([128, 1], F32)
            nc.gpsimd.memset(cmag[:], CMAGIC)

            # Warm-up AllReduce: the first collective's barrier absorbs the
            # cross-core launch skew (~40-50us on HW). Firing a dummy one at
            # t~0 hides that skew under pass 1, so the real AllReduce later
            # only pays the mesh latency.
            nc.gpsimd.collective_compute(
                "AllReduce", mybir.AluOpType.add,
                replica_groups=[list(range(N_CORES))],
                ins=[cc_warm[:]], outs=[cc_warm[:]])

            # The PE power governor duty-cycles the tensor engine to 4/8 for a
            # fixed ~270us wall-clock window after it first becomes active
            # (HAM events in the profile). Left alone, that window lands on
            # the real matmul phase and costs ~110us. Dummy fp32 matmuls on
            # data already in SBUF, paced by the prologue's chunk arrivals,
            # trigger and ride out the ramp while the PE would be idle anyway.
            def dummy_mm(src, n):
                pass

            # ---- pass 1: abs-sum of the fp32 weight slice ----
            # DMA alternates the two HWDGE queues (the gpsimd soft-DGE queue
            # must stay empty: its traffic contends with the collective);
            # the vector reduces hide underneath.
            for c in range(NWC):
                for j in range(NJ):
                    i = NJ * c + j
                    wchunk = w32_pool.tile([128, CW], F32, tag="w32")
                    deng = nc.scalar if i % 2 == 0 else nc.sync
                    deng.dma_start(
                        wchunk[:], w_d[c * 128:(c + 1) * 128, j * CW:(j + 1) * CW])
                    nc.vector.tensor_reduce(
                        partials[:, i:i + 1], wchunk[:],
                        axis=mybir.AxisListType.X,
                        op=mybir.AluOpType.add, apply_absolute_value=True)
                    dummy_mm(wchunk, 2)

            # partials -> one scalar -> AllReduce across the 8 cores
            psum1 = small.tile([128, 1], F32)
            nc.vector.tensor_reduce(psum1[:], partials[:], axis=mybir.AxisListType.X,
                                    op=mybir.AluOpType.add)
            lsum = small.tile([1, 1], F32)
            nc.gpsimd.tensor_reduce(lsum[:], psum1[:], axis=mybir.AxisListType.C,
                                    op=mybir.AluOpType.add)
            nc.gpsimd.dma_start(cc_buf[:], lsum[:])
            nc.gpsimd.collective_compute(
                "AllReduce", mybir.AluOpType.add,
                replica_groups=[list(range(N_CORES))],
                ins=[cc_buf[:]], outs=[cc_buf[:]])

            # ---- x pipeline ----
            xqTs = {}
            gvs = {}
            evecs = {}

            def x_tile_prep(t, prime=False, after=None):
                xh = []
                gpart = small.tile([128, 2], F32, tag=f"gp{t % 8}",
                                   name=f"gp_{t}")
                for h in range(2):
                    xt = x32_pool.tile([128, DH], F32, tag="x32", name=f"x_{t}_{h}")
                    xdma = nc.sync.dma_start(
                        xt[:], x_d[t * 128:(t + 1) * 128, h * DH:(h + 1) * DH])
                    if after is not None:
                        tile.add_dep_helper(xdma.ins, after, sync=True,
                                            reason="x loads yield to og0 prep")
                    nc.vector.tensor_reduce(gpart[:, h:h + 1], xt[:],
                                            axis=mybir.AxisListType.X,
                                            op=mybir.AluOpType.max,
                                            apply_absolute_value=True)
                    if prime:
                        dummy_mm(xt, 2)
                    xh.append(xt)
                gv = small.tile([128, 2], F32, tag=f"gv{t % 8}", name=f"gv_{t}")
                gam, qs = gv[:, 0:1], gv[:, 1:2]
                nc.vector.tensor_reduce(gam, gpart[:], axis=mybir.AxisListType.X,
                                        op=mybir.AluOpType.max)
                nc.vector.tensor_scalar_add(qs, gam, EPS)
                nc.vector.reciprocal(qs, qs)
                nc.vector.tensor_scalar_mul(qs, qs, 127.0)
                gvs[t] = gv

                xqT = xqT_pool.tile([128, ND, 128], BF16, tag="xqT", name=f"xqT_{t}")
                for h in range(2):
                    xq16 = xq16_pool.tile([128, DH], BF16, tag="xq16")
                    for q in range(2):
                        sl = slice(q * 1024, (q + 1) * 1024)
                        x1 = xt1_pool.tile([128, 1024], F32, tag="xt1")
                        nc.scalar.activation(x1[:], xh[h][:, sl],
                                             mybir.ActivationFunctionType.Identity,
                                             bias=cmag[:], scale=qs)
                        nc.vector.tensor_scalar_add(xq16[:, sl], x1[:], -CMAGIC)
                    nc.sync.dma_start_transpose(
                        out=xqT[:, h * NDH:(h + 1) * NDH, :], in_=xq16[:])
                xqTs[t] = xqT

            def evec_prep(t):
                evec = small.tile([128, 1], F32, tag=f"ev{t % 8}", name=f"ev_{t}")
                nc.vector.tensor_tensor(out=evec[:], in0=gvs[t][:, 0:1],
                                        in1=sw127_b,
                                        op=mybir.AluOpType.mult)
                evecs[t] = evec

            # prefetch NPREF token tiles; their loads queue behind pass-1's
            # ---- pass 2 re-read DMA issues: og0's first half ----
            # og0-h0 goes on the scalar HWDGE queue right after pass 1's even
            # chunks (4 issues fit the w32 ring with no pool wait, so no
            # head-of-line risk) and lands while the collective runs.
            p2chunks = {}

            p2order = [(4 * g + cg, j)
                       for g in range(NOG) for j in range(NJ) for cg in range(4)]

            def p2_issue(eng, c, j, prime=False):
                wchunk = w32_pool.tile([128, CW], F32, tag="w32",
                                       name=f"w2_{c}_{j}")
                eng.dma_start(
                    wchunk[:], w_d[c * 128:(c + 1) * 128, j * CW:(j + 1) * CW])
                if prime:
                    dummy_mm(wchunk, 2)
                p2chunks[(c, j)] = wchunk

            N_EARLY = 10  # fits the w32 ring: no pool wait on the scalar queue
            for c, j in p2order[:N_EARLY]:
                p2_issue(nc.scalar, c, j, prime=True)

            # x prefetch t0/t1 only before the AllReduce tail: their loads
            # queue behind pass-1's sync half, and their vector / scalar work
            # fills the collective's latency window. t2/t3 are prepped after
            # og0's ternarize so og0 work never queues behind them.
            for t in range(2):
                x_tile_prep(t, prime=True)

            # ---- AllReduce tail: one broadcast DMA straight from cc_buf,
            # scales computed redundantly on all 128 partitions ----
            gsum_b = small.tile([128, 1], F32)
            nc.gpsimd.dma_start(gsum_b[:], cc_buf[:].partition_broadcast(128))
            sc2b = small.tile([128, 2], F32)
            nc.vector.tensor_scalar(sc2b[:, 0:1], gsum_b[:],
                                    1.0 / (D_OUT * D_IN), EPS,
                                    op0=mybir.AluOpType.mult,
                                    op1=mybir.AluOpType.add)
            nc.vector.reciprocal(sc2b[:, 0:1], sc2b[:, 0:1])
            nc.vector.tensor_scalar_mul(sc2b[:, 1:2], gsum_b[:],
                                        1.0 / (D_OUT * D_IN * 127.0))
            rsw_b = sc2b[:, 0:1]
            sw127_b = sc2b[:, 1:2]
            for t in range(2):
                evec_prep(t)

            # Remaining re-reads on the gpsimd soft-DGE queue after the
            # collective: pool throttling stalls only that queue.
            for c, j in p2order[N_EARLY:]:
                p2_issue(nc.gpsimd, c, j)

            # ---- pass 2 compute: ternarize, transpose, cast into the slab.
            # og0 first; the x t2/t3 prep is emitted after it so og0's acts
            # and transposes are not queued behind x traffic. ----
            def p2_compute(c, j):
                g, cg = c // 4, c % 4
                wchunk = p2chunks[(c, j)]
                tern = wtern_pool.tile([128, CW], BF16, tag="wtern")
                tw = wt1_pool.tile([128, CW], F32, tag="wt1")
                nc.scalar.activation(tw[:], wchunk[:],
                                     mybir.ActivationFunctionType.Identity,
                                     bias=cmag[:], scale=rsw_b)
                nc.vector.tensor_scalar(tern[:], tw[:], -CMAGIC, -1.0,
                                        op0=mybir.AluOpType.add,
                                        op1=mybir.AluOpType.max)
                nc.vector.tensor_scalar_min(tern[:], tern[:], 1.0)
                wtT = wtT_pool.tile([128, NKJ, 128], BF16, tag="wtT")
                nc.sync.dma_start_transpose(out=wtT[:], in_=tern[:])
                return nc.vector.tensor_copy(
                    w8[g][:, j, :, cg * 128:(cg + 1) * 128], wtT[:])

            og0_done = None
            for c, j in p2order[:16]:
                og0_done = p2_compute(c, j)
            # x t2/t3 prep explicitly yields to og0's chain: the scheduler's
            # DMA model is optimistic about x load latency and would
            # otherwise slot this traffic ahead of og0 on the scalar/sync
            # queues, pushing the first real matmul out by ~70us.
            for t in range(2, NPREF):
                x_tile_prep(t, after=og0_done.ins)
                evec_prep(t)
            for c, j in p2order[16:]:
                p2_compute(c, j)

            # ---- matmuls ----
            def mm_group(t, og):
                acc = psum_pool.tile([128, 512], F32, tag=f"acc{og}",
                                     name=f"acc_{t}_{og}")
                xqT = xqTs[t]
                for k in range(ND):
                    j, kk = k // NKJ, k % NKJ
                    nc.tensor.matmul(acc[:], xqT[:, k, :],
                                     w8[og][:, j, kk, :],
                                     start=(k == 0), stop=(k == ND - 1))
                ot = outs_pool.tile([128, 512], F32, tag="outs")
                nc.scalar.activation(ot[:], acc[:],
                                     mybir.ActivationFunctionType.Copy,
                                     bias=0.0, scale=evecs[t][:])
                nc.scalar.dma_start(
                    out_d[t * 128:(t + 1) * 128, og * 512:(og + 1) * 512], ot[:])

            # og-outer over the prefetched tiles so matmuls start on w8[og=0]
            # while pass 2 is still ternarizing the later groups
            for og in range(NOG):
                for t in range(NPREF):
                    mm_group(t, og)
            # steady state: t-outer
            for t in range(NPREF, NTOK):
                x_tile_prep(t)
                evec_prep(t)
                for og in range(NOG):
                    mm_group(t, og)

    _split_multi_waits(nc)
    return nc


_NC_CACHE = None


def kernel(x: np.ndarray, weight: np.ndarray, _want_profile=False, **_kw):
    global _NC_CACHE
    assert x.shape == (B, T, D_IN) and weight.shape == (D_OUT, D_IN)
    x_flat = np.ascontiguousarray(x.reshape(TOK, D_IN), dtype=np.float32)
    w = np.ascontiguousarray(weight, dtype=np.float32)

    if _NC_CACHE is None:
        _NC_CACHE = build_bitlinear_nc()
    nc = _NC_CACHE

    in_maps = [
        {"x": x_flat, "weight": w[c * OPC:(c + 1) * OPC, :]}
        for c in range(N_CORES)
    ]
    res = run_bass_kernel_spmd(nc, in_maps, list(range(N_CORES)),
                               trace=bool(_want_profile))
    out = np.concatenate([res.results[c]["out"] for c in range(N_CORES)], axis=1)
    out = out.reshape(B, T, D_OUT)
    if _want_profile:
        return out, res
    return out
